# revision 5
# baseline (speedup 1.0000x reference)
"""Trainium2 Bass kernel for the CEVAE-guide multi-head MLP (moe_routing).

Strategy:
  - Pure data parallel: batch B=16384 split across 8 NeuronCores (2048 each).
  - Host-side MoE routing for the per-treatment heads: each core's columns
    are grouped by treatment id into fixed-size slots (same layout on every
    core -> one SPMD program with static offsets). The z head then computes
    only the selected expert per column (contiguous slices), 7x less work
    than the dense reference.
  - Activations kept feature-on-partition ([feat, batch]) so the matmul
    chain needs no transposes; host pre-transposes x.
  - Matmuls run as float32r (full PE rate for fp32 data, ~1e-4/layer err).
  - ELU = Relu(x) + (min(Exp(x),1) - 1), biases folded into the ACT pass.
  - y/d head treatment-selection (2 of 14 raw columns) + their tiny scalar
    nonlinearities run on host; all heavy math runs on device.
"""

import os
import sys

import numpy as np

sys.path.insert(0, "/opt/trn_rl_repo")

import concourse.bass as bass  # noqa: E402
import concourse.mybir as mybir  # noqa: E402
import concourse.tile as tile  # noqa: E402
from concourse import bacc  # noqa: E402
from concourse.bass_utils import run_bass_kernel_spmd  # noqa: E402

B, F, H, L, K = 16384, 1024, 1024, 256, 7
NCORES = 8
DT = mybir.dt.float32
R = mybir.dt.float32 if os.environ.get("CEVAE_FP32") else mybir.dt.float32r
AF = mybir.ActivationFunctionType
ALU = mybir.AluOpType

_PROG_CACHE: dict = {}


# ----------------------------------------------------------------------------
# Routing plan
# ----------------------------------------------------------------------------
def _plan_routing(t: np.ndarray):
    """Group batch rows by treatment into per-core column slots.

    Every core gets the same slot layout (n_slots per expert job), so a
    single SPMD program with compile-time offsets serves all cores.
    """
    jobs = []  # (expert, n_slots, [per-core row-index arrays])
    for k in range(K):
        idx = np.nonzero(t == k)[0]
        need = max(1, -(-len(idx) // NCORES))
        nparts = max(1, -(-need // 512))  # keep each job's slots <= 512
        for part in np.array_split(idx, nparts):
            per_core = np.array_split(part, NCORES)
            n_slots = max(256, max(len(p) for p in per_core))
            n_slots = min((n_slots + 1) // 2 * 2, 512)  # even (fp32r ISA)
            jobs.append((k, n_slots, per_core))

    # balance jobs into two halves (expert-set split) by slot count
    order = sorted(range(len(jobs)), key=lambda j: -jobs[j][1])
    tot, sets = [0, 0], [[], []]
    for j in order:
        s = 0 if tot[0] <= tot[1] else 1
        sets[s].append(j)
        tot[s] += jobs[j][1]

    layout = []  # job ids in column order
    half_jobs = [[], []]  # (expert, local_off, n_slots)
    half_sizes = []
    for s in (0, 1):
        off = 0
        for j in sorted(sets[s]):
            k, n_slots, _ = jobs[j]
            half_jobs[s].append((k, off, n_slots))
            layout.append(j)
            off += n_slots
        half_sizes.append(off)
    b_dev = sum(half_sizes)

    perm = np.zeros((NCORES, b_dev), dtype=np.int64)
    valid = np.zeros((NCORES, b_dev), dtype=bool)
    for c in range(NCORES):
        off = 0
        for j in layout:
            _, n_slots, per_core = jobs[j]
            rows = per_core[c]
            n = len(rows)
            perm[c, off:off + n] = rows
            valid[c, off:off + n] = True
            filler = rows[0] if n else 0
            perm[c, off + n:off + n_slots] = filler
            off += n_slots

    return {
        "b_dev": b_dev,
        "half_sizes": tuple(half_sizes),
        "half_jobs": (tuple(half_jobs[0]), tuple(half_jobs[1])),
        "perm": perm,
        "valid": valid,
    }


def _chunk_groups(bh: int):
    """Split a half's `bh` columns into PSUM groups (<=1024 wide, <=2 banks)
    of matmul subchunks. Each matmul's output must stay inside one 2KB PSUM
    bank (512 fp32), so a 2-sub group is always [512, tail]; widths are even
    (fp32r ISA) and kept >=256 where possible (fp32r full-rate threshold).
    """
    assert bh % 2 == 0
    q, r = divmod(bh, 512)
    if r == 0:
        subs = [512] * q
    elif r >= 256 or q == 0:
        subs = [512] * q + [r]
    else:  # split the last 512+r into two balanced subs in [256, 384]
        s1 = (512 + r) // 4 * 2
        subs = [512] * (q - 1) + [s1, 512 + r - s1]
    groups = []  # (group_off, group_width, [(sub_off, sub_w), ...])
    off = 0
    i = 0
    while i < len(subs):
        # pair only when the first sub is exactly 512 (bank-aligned split)
        if subs[i] == 512 and i + 1 < len(subs) and subs[i + 1] <= 512:
            take = [512, subs[i + 1]]
        else:
            take = [subs[i]]
        gw = sum(take)
        soff, ss = 0, []
        for w in take:
            ss.append((soff, w))
            soff += w
        groups.append((off, gw, ss))
        off += gw
        i += len(take)
    return groups


# ----------------------------------------------------------------------------
# Program builder
# ----------------------------------------------------------------------------
def _build_program(sig):
    b_dev, half_sizes, half_jobs = sig
    nc = bacc.Bacc("TRN2", target_bir_lowering=False, debug=False,
                   num_devices=NCORES)

    din = lambda n, s, d=R: nc.dram_tensor(n, list(s), d, kind="ExternalInput")
    dout = lambda n, s: nc.dram_tensor(n, list(s), DT, kind="ExternalOutput")

    xT = din("xT", (F, b_dev))
    ydm = din("yd", (2, b_dev))
    tw0, tw1, tw2 = din("tw0", (F, L)), din("tw1", (L, L)), din("tw2", (L, K))
    yw0, yw1 = din("yw0", (F, H)), din("yw1", (H, H))
    dw0, dw1 = din("dw0", (F, H)), din("dw1", (H, H))
    zw0, zw1 = din("zw0", (F + 2, H)), din("zw1", (H, H))
    yhw, dhw = din("yhw", (H, 2 * K)), din("dhw", (H, 2 * K))
    zhW = din("zhW", (K, H, 2 * L))
    # biases (host-prepacked layouts)
    tb0 = din("tb0", (128, 2), DT)
    tb1 = din("tb1", (128, 2), DT)
    tb2 = din("tb2", (K, 1), DT)
    yb0, yb1 = din("yb0", (128, 8), DT), din("yb1", (128, 8), DT)
    db0, db1 = din("db0", (128, 8), DT), din("db1", (128, 8), DT)
    zb0, zb1 = din("zb0", (128, 8), DT), din("zb1", (128, 8), DT)
    yhb, dhb = din("yhb", (2 * K, 1), DT), din("dhb", (2 * K, 1), DT)
    zhb = din("zhb", (128, 4 * K), DT)  # col = k*4 + mj

    tlog = dout("tlog", (K, b_dev))
    yall = dout("yall", (2 * K, b_dev))
    dall = dout("dall", (2 * K, b_dev))
    zloc = dout("zloc", (L, b_dev))
    zscale = dout("zscale", (L, b_dev))

    with tile.TileContext(nc) as tc:
        with (
            tc.tile_pool(name="const", bufs=1) as pc,
            tc.tile_pool(name="px", bufs=2) as px,
            tc.tile_pool(name="pact", bufs=1) as pact,
            tc.tile_pool(name="pw", bufs=6) as pw,
            tc.tile_pool(name="pwyd", bufs=2) as pwyd,
            tc.tile_pool(name="pe", bufs=3) as pe_,
            tc.tile_pool(name="po", bufs=4) as po,
            tc.tile_pool(name="pyd", bufs=2) as pyd,
            tc.tile_pool(name="pps", bufs=3, space="PSUM") as pps,
        ):
            # ---- constants ----
            def cload(drm, shape, tag):
                t_ = pc.tile(list(shape), DT, tag=tag)
                nc.sync.dma_start(t_[:], drm[:, :])
                return t_

            tb0t = cload(tb0, (128, 2), "tb0")
            tb1t = cload(tb1, (128, 2), "tb1")
            tb2t = cload(tb2, (K, 1), "tb2")
            yb0t = cload(yb0, (128, 8), "yb0")
            yb1t = cload(yb1, (128, 8), "yb1")
            db0t = cload(db0, (128, 8), "db0")
            db1t = cload(db1, (128, 8), "db1")
            zb0t = cload(zb0, (128, 8), "zb0")
            zb1t = cload(zb1, (128, 8), "zb1")
            yhbt = cload(yhb, (2 * K, 1), "yhb")
            dhbt = cload(dhb, (2 * K, 1), "dhb")
            zhbt = cload(zhb, (128, 4 * K), "zhb")
            # y/d head weights, resident
            yhwt = pc.tile([128, 8, 2 * K], R, tag="yhwt")
            dhwt = pc.tile([128, 8, 2 * K], R, tag="dhwt")
            for ki in range(8):
                nc.sync.dma_start(yhwt[:, ki, :], yhw[128 * ki:128 * (ki + 1), :])
                nc.sync.dma_start(dhwt[:, ki, :], dhw[128 * ki:128 * (ki + 1), :])

            def elu_post(ps, gw, bias, dst, extra_min=None, part=128):
                e = pe_.tile([part, gw], DT, tag="e")
                nc.scalar.activation(dst, ps[:, :gw], AF.Relu, bias=bias)
                nc.scalar.activation(e[:, :], ps[:, :gw], AF.Exp, bias=bias)
                nc.vector.tensor_scalar(e[:, :], e[:, :], 1.0, -1.0,
                                        ALU.min, ALU.add)
                nc.vector.tensor_tensor(dst, dst, e[:, :], ALU.add)
                if extra_min is not None:
                    nc.vector.tensor_scalar_min(dst, dst, extra_min)

            def emit_layer(src, kt, wdram, wroff, mt, bias_t, dst, groups,
                           extra=None):
                """dst(mi) -> AP [128, BH]; src(ki) -> AP [128, BH]."""
                for mi in range(mt):
                    wt = pw.tile([128, kt, 128], R, tag="W")
                    for ki in range(kt):
                        nc.sync.dma_start(
                            wt[:, ki, :],
                            wdram[wroff + 128 * ki:wroff + 128 * (ki + 1),
                                  128 * mi:128 * (mi + 1)])
                    ex = extra(mi) if extra is not None else None
                    for (goff, gw, ss) in groups:
                        ps = pps.tile([128, gw], DT, tag="acc")
                        for (soff, sw) in ss:
                            a = goff + soff
                            for ki in range(kt):
                                nc.tensor.matmul(
                                    ps[:, soff:soff + sw], wt[:, ki, :],
                                    src(ki)[:, a:a + sw],
                                    start=(ki == 0),
                                    stop=(ki == kt - 1 and ex is None))
                            if ex is not None:
                                lhs, rhs = ex
                                nc.tensor.matmul(ps[:, soff:soff + sw], lhs,
                                                 rhs[:, a:a + sw],
                                                 start=False, stop=True)
                        elu_post(ps, gw, bias_t[:, mi:mi + 1],
                                 dst(mi)[:, goff:goff + gw])

            def emit_head(src, kt, wtile, p_out, bias_col, out_dram, hoff,
                          groups, mode):
                """Small-head layer: p_out<=128 output features, one m-tile."""
                for (goff, gw, ss) in groups:
                    ps = pps.tile([p_out, gw], DT, tag="acc")
                    for (soff, sw) in ss:
                        a = goff + soff
                        for ki in range(kt):
                            nc.tensor.matmul(ps[:, soff:soff + sw],
                                             wtile(ki), src(ki)[:, a:a + sw],
                                             start=(ki == 0),
                                             stop=(ki == kt - 1))
                    ot = po.tile([p_out, gw], DT, tag="oS")
                    if mode == "eluclip":
                        elu_post(ps, gw, bias_col, ot[:, :], extra_min=10.0,
                                 part=p_out)
                    else:  # raw + bias
                        nc.scalar.activation(ot[:, :], ps[:, :gw], AF.Identity,
                                             bias=bias_col)
                    nc.sync.dma_start(
                        out_dram[0:p_out, hoff + goff:hoff + goff + gw],
                        ot[:, :])

            hoff = 0
            for hf in (0, 1):
                bh = half_sizes[hf]
                groups = _chunk_groups(bh)
                xh = px.tile([128, 8, bh], R, tag="x")
                for ki in range(8):
                    nc.sync.dma_start(xh[:, ki, :],
                                      xT[128 * ki:128 * (ki + 1),
                                         hoff:hoff + bh])
                ydh = pyd.tile([2, bh], R, tag="yd")
                nc.sync.dma_start(ydh[:, :], ydm[:, hoff:hoff + bh])

                xsrc = lambda ki: xh[:, ki, :]

                # ---- t branch ----
                h1 = pact.tile([128, 2, bh], R, tag="hA")
                emit_layer(xsrc, 8, tw0, 0, 2, tb0t,
                           lambda mi: h1[:, mi, :], groups)
                h2 = pact.tile([128, 2, bh], R, tag="hB")
                emit_layer(lambda ki: h1[:, ki, :], 2, tw1, 0, 2, tb1t,
                           lambda mi: h2[:, mi, :], groups)
                tw2t = pwyd.tile([128, 2, K], R, tag="tw2")
                for ki in range(2):
                    nc.sync.dma_start(tw2t[:, ki, :],
                                      tw2[128 * ki:128 * (ki + 1), :])
                emit_head(lambda ki: h2[:, ki, :], 2,
                          lambda ki: tw2t[:, ki, :], K, tb2t[:, 0:1],
                          tlog, hoff, groups, "eluclip")

                # ---- y branch ----
                hy1 = pact.tile([128, 8, bh], R, tag="hA")
                emit_layer(xsrc, 8, yw0, 0, 8, yb0t,
                           lambda mi: hy1[:, mi, :], groups)
                hy2 = pact.tile([128, 8, bh], R, tag="hB")
                emit_layer(lambda ki: hy1[:, ki, :], 8, yw1, 0, 8, yb1t,
                           lambda mi: hy2[:, mi, :], groups)
                emit_head(lambda ki: hy2[:, ki, :], 8,
                          lambda ki: yhwt[:, ki, :], 2 * K, yhbt[:, 0:1],
                          yall, hoff, groups, "raw")

                # ---- d branch ----
                hd1 = pact.tile([128, 8, bh], R, tag="hA")
                emit_layer(xsrc, 8, dw0, 0, 8, db0t,
                           lambda mi: hd1[:, mi, :], groups)
                hd2 = pact.tile([128, 8, bh], R, tag="hB")
                emit_layer(lambda ki: hd1[:, ki, :], 8, dw1, 0, 8, db1t,
                           lambda mi: hd2[:, mi, :], groups)
                emit_head(lambda ki: hd2[:, ki, :], 8,
                          lambda ki: dhwt[:, ki, :], 2 * K, dhbt[:, 0:1],
                          dall, hoff, groups, "raw")

                # ---- z branch ----
                def z_extra(mi):
                    wyd = pwyd.tile([2, 128], R, tag="wyd")
                    nc.sync.dma_start(wyd[:, :],
                                      zw0[0:2, 128 * mi:128 * (mi + 1)])
                    return (wyd[:, :], ydh)

                hz1 = pact.tile([128, 8, bh], R, tag="hA")
                emit_layer(xsrc, 8, zw0, 2, 8, zb0t,
                           lambda mi: hz1[:, mi, :], groups, extra=z_extra)
                hz2 = pact.tile([128, 8, bh], R, tag="hB")
                emit_layer(lambda ki: hz1[:, ki, :], 8, zw1, 0, 8, zb1t,
                           lambda mi: hz2[:, mi, :], groups)

                # ---- z head (routed) ----
                for (k, loff, n) in half_jobs[hf]:
                    for mj in range(4):
                        wt = pw.tile([128, 8, 128], R, tag="W")
                        for ki in range(8):
                            nc.sync.dma_start(
                                wt[:, ki, :],
                                zhW[k, 128 * ki:128 * (ki + 1),
                                    128 * mj:128 * (mj + 1)])
                        ps = pps.tile([128, n], DT, tag="acc")
                        for ki in range(8):
                            nc.tensor.matmul(ps[:, :],
                                             wt[:, ki, :],
                                             hz2[:, ki, loff:loff + n],
                                             start=(ki == 0), stop=(ki == 7))
                        bias = zhbt[:, 4 * k + mj:4 * k + mj + 1]
                        ot = po.tile([128, n], DT, tag="oS")
                        if mj < 2:
                            nc.scalar.activation(ot[:, :], ps[:, :],
                                                 AF.Identity, bias=bias)
                            nc.vector.tensor_scalar(ot[:, :], ot[:, :],
                                                    -100.0, 100.0,
                                                    ALU.max, ALU.min)
                            dst = zloc
                            r0 = 128 * mj
                        else:
                            # softplus(x) = relu(x) + ln(1 + exp(-|x|))
                            a1 = pe_.tile([128, n], DT, tag="e")
                            nc.scalar.activation(a1[:, :], ps[:, :], AF.Abs,
                                                 bias=bias)
                            a2 = pe_.tile([128, n], DT, tag="e")
                            nc.scalar.activation(a2[:, :], a1[:, :], AF.Exp,
                                                 scale=-1.0)
                            a3 = pe_.tile([128, n], DT, tag="e")
                            nc.scalar.activation(a3[:, :], a2[:, :], AF.Ln,
                                                 bias=1.0)
                            nc.scalar.activation(ot[:, :], ps[:, :], AF.Relu,
                                                 bias=bias)
                            nc.vector.tensor_tensor(ot[:, :], ot[:, :],
                                                    a3[:, :], ALU.add)
                            nc.vector.tensor_scalar(ot[:, :], ot[:, :],
                                                    0.001, 100.0,
                                                    ALU.add, ALU.min)
                            dst = zscale
                            r0 = 128 * (mj - 2)
                        nc.sync.dma_start(
                            dst[r0:r0 + 128, hoff + loff:hoff + loff + n],
                            ot[:, :])
                hoff += bh

    nc.compile()
    return nc


def _get_program(sig):
    if sig not in _PROG_CACHE:
        _PROG_CACHE[sig] = _build_program(sig)
    return _PROG_CACHE[sig]


# ----------------------------------------------------------------------------
# Host-side glue
# ----------------------------------------------------------------------------
def _softplus64(x):
    x = x.astype(np.float64)
    return np.log1p(np.exp(-np.abs(x))) + np.maximum(x, 0.0)


def _f32(a):
    return np.ascontiguousarray(np.asarray(a), dtype=np.float32)


def make_in_maps(inputs, plan):
    x = _f32(inputs["x"])
    y = _f32(inputs["y"])
    d = _f32(inputs["d"])
    perm = plan["perm"]

    def pack_bias(v, rows=128):
        v = _f32(v).reshape(-1)
        if len(v) % rows == 0 and len(v) >= rows:
            return np.ascontiguousarray(v.reshape(-1, rows).T)
        return v.reshape(-1, 1)

    shared = {
        "tw0": _f32(inputs["tw0"]), "tw1": _f32(inputs["tw1"]),
        "tw2": _f32(inputs["tw2"]),
        "yw0": _f32(inputs["yw0"]), "yw1": _f32(inputs["yw1"]),
        "dw0": _f32(inputs["dw0"]), "dw1": _f32(inputs["dw1"]),
        "zw0": _f32(inputs["zw0"]), "zw1": _f32(inputs["zw1"]),
        "yhw": _f32(np.transpose(np.asarray(inputs["yhW"]), (1, 0, 2))
                    .reshape(H, 2 * K)),
        "dhw": _f32(np.transpose(np.asarray(inputs["dhW"]), (1, 0, 2))
                    .reshape(H, 2 * K)),
        "zhW": _f32(inputs["zhW"]),
        "tb0": pack_bias(inputs["tb0"]), "tb1": pack_bias(inputs["tb1"]),
        "tb2": pack_bias(inputs["tb2"]),
        "yb0": pack_bias(inputs["yb0"]), "yb1": pack_bias(inputs["yb1"]),
        "db0": pack_bias(inputs["db0"]), "db1": pack_bias(inputs["db1"]),
        "zb0": pack_bias(inputs["zb0"]), "zb1": pack_bias(inputs["zb1"]),
        "yhb": _f32(inputs["yhb"]).reshape(2 * K, 1),
        "dhb": _f32(inputs["dhb"]).reshape(2 * K, 1),
        "zhb": np.ascontiguousarray(
            _f32(inputs["zhb"]).reshape(K, 4, 128).transpose(2, 0, 1)
            .reshape(128, 4 * K)),
    }
    in_maps = []
    for c in range(NCORES):
        rows = perm[c]
        m = dict(shared)
        m["xT"] = np.ascontiguousarray(x[rows].T)
        m["yd"] = np.ascontiguousarray(
            np.stack([y[rows], d[rows]], axis=0))
        in_maps.append(m)
    return in_maps


def assemble(results, plan, t):
    out = np.empty((B, K + 4 + 2 * L), dtype=np.float32)
    perm, valid = plan["perm"], plan["valid"]
    for c in range(NCORES):
        r = results[c]
        cols = np.nonzero(valid[c])[0]
        rows = perm[c][cols]
        out[rows, 0:K] = r["tlog"][:, cols].T
        tr = t[rows].astype(np.int64)
        ar = np.arange(len(cols))
        for name, o in (("yall", K), ("dall", K + 2)):
            a = r[name][:, cols]
            loc = np.clip(a[2 * tr, ar], -1e6, 1e6)
            scale = np.minimum(_softplus64(a[2 * tr + 1, ar]) + 1e-3, 1e6)
            out[rows, o] = loc
            out[rows, o + 1] = scale.astype(np.float32)
        out[rows, K + 4:K + 4 + L] = r["zloc"][:, cols].T
        out[rows, K + 4 + L:] = r["zscale"][:, cols].T
    return out


def kernel(**inputs) -> np.ndarray:
    t = np.asarray(inputs["t"]).astype(np.int32)
    plan = _plan_routing(t)
    sig = (plan["b_dev"], plan["half_sizes"], plan["half_jobs"])
    nc = _get_program(sig)
    in_maps = make_in_maps(inputs, plan)
    res = run_bass_kernel_spmd(nc, in_maps, list(range(NCORES)))
    return assemble(res.results, plan, t)


# revision 17
# speedup vs baseline: 1.6227x; 1.6227x over previous
"""Trainium2 Bass kernel for the CEVAE-guide multi-head MLP (moe_routing).

Strategy:
  - Pure data parallel: batch B=16384 split across 8 NeuronCores (2048 each).
  - Host-side MoE routing for the per-treatment heads: each core's columns
    are grouped by treatment id into fixed-size slots (same layout on every
    core -> one SPMD program with static offsets). The z head then computes
    only the selected expert per column (contiguous slices), 7x less work
    than the dense reference.
  - Activations kept feature-on-partition ([feat, batch]) so the matmul
    chain needs no transposes; host pre-transposes x.
  - Matmuls run as float32r (full PE rate for fp32 data, ~1e-4/layer err).
  - ELU = Relu(x) + (min(Exp(x),1) - 1), biases folded into the ACT pass.
  - y/d head treatment-selection (2 of 14 raw columns) + their tiny scalar
    nonlinearities run on host; all heavy math runs on device.
"""

import os
import sys

import numpy as np

sys.path.insert(0, "/opt/trn_rl_repo")

import concourse.bass as bass  # noqa: E402
import concourse.mybir as mybir  # noqa: E402
import concourse.tile as tile  # noqa: E402
from concourse import bacc  # noqa: E402
from concourse.bass_utils import run_bass_kernel_spmd  # noqa: E402

B, F, H, L, K = 16384, 1024, 1024, 256, 7
NCORES = 8
DT = mybir.dt.float32
R = mybir.dt.float32 if os.environ.get("CEVAE_FP32") else mybir.dt.float32r
AF = mybir.ActivationFunctionType
ALU = mybir.AluOpType

_PROG_CACHE: dict = {}

# Every ACT function this kernel uses (Relu, Exp, Abs, Ln, Identity) lives in
# the natural_log_exp_and_others table; restricting the table-load pass to it
# yields a single LoadActFuncSet instead of exp<->ln thrash (~2.7us each).
_ACT_KEEP = "natural_log_exp_and_others"


def _patch_act_tables():
    from concourse import bacc as _bacc_mod
    orig = _bacc_mod.get_activation_tables
    if getattr(orig, "_cevae_patched", False):
        return
    def patched(arch):
        tabs = orig(arch)
        if _ACT_KEEP in tabs:
            tabs = {name: (funcs if name == _ACT_KEEP else set())
                    for name, funcs in tabs.items()}
        return tabs
    patched._cevae_patched = True
    _bacc_mod.get_activation_tables = patched


# ----------------------------------------------------------------------------
# Routing plan
# ----------------------------------------------------------------------------
def _plan_routing(t: np.ndarray):
    """Group batch rows by treatment into per-core column slots.

    Every core gets the same slot layout (n_slots per expert job), so a
    single SPMD program with compile-time offsets serves all cores.
    """
    jobs = []  # (expert, n_slots, [per-core row-index arrays])
    for k in range(K):
        idx = np.nonzero(t == k)[0]
        need = max(1, -(-len(idx) // NCORES))
        nparts = max(1, -(-need // 512))  # keep each job's slots <= 512
        for part in np.array_split(idx, nparts):
            per_core = np.array_split(part, NCORES)
            n_slots = max(256, max(len(p) for p in per_core))
            n_slots = min((n_slots + 1) // 2 * 2, 512)  # even (fp32r ISA)
            jobs.append((k, n_slots, per_core))

    # balance jobs into two halves (expert-set split) by slot count
    order = sorted(range(len(jobs)), key=lambda j: -jobs[j][1])
    tot, sets = [0, 0], [[], []]
    for j in order:
        s = 0 if tot[0] <= tot[1] else 1
        sets[s].append(j)
        tot[s] += jobs[j][1]

    layout = []  # job ids in column order
    half_jobs = [[], []]  # (expert, local_off, n_slots)
    half_sizes = []
    for s in (0, 1):
        off = 0
        for j in sorted(sets[s]):
            k, n_slots, _ = jobs[j]
            half_jobs[s].append((k, off, n_slots))
            layout.append(j)
            off += n_slots
        half_sizes.append(off)
    b_dev = sum(half_sizes)

    perm = np.zeros((NCORES, b_dev), dtype=np.int64)
    valid = np.zeros((NCORES, b_dev), dtype=bool)
    for c in range(NCORES):
        off = 0
        for j in layout:
            _, n_slots, per_core = jobs[j]
            rows = per_core[c]
            n = len(rows)
            perm[c, off:off + n] = rows
            valid[c, off:off + n] = True
            filler = rows[0] if n else 0
            perm[c, off + n:off + n_slots] = filler
            off += n_slots

    return {
        "b_dev": b_dev,
        "half_sizes": tuple(half_sizes),
        "half_jobs": (tuple(half_jobs[0]), tuple(half_jobs[1])),
        "perm": perm,
        "valid": valid,
    }


def _chunk_groups(bh: int):
    """Split a half's `bh` columns into PSUM groups (<=1536 wide, <=3 banks)
    of matmul subchunks. Each matmul's output must stay inside one 2KB PSUM
    bank (512 fp32), so every non-final sub in a group is exactly 512;
    widths are even (fp32r ISA) and kept >=256 where possible (fp32r
    full-rate threshold)."""
    assert bh % 2 == 0
    q, r = divmod(bh, 512)
    if r == 0:
        subs = [512] * q
    elif r >= 256 or q == 0:
        subs = [512] * q + [r]
    else:  # split the last 512+r into two balanced subs in [256, 384]
        s1 = (512 + r) // 4 * 2
        subs = [512] * (q - 1) + [s1, 512 + r - s1]
    groups = []  # (group_off, group_width, [(sub_off, sub_w), ...])
    off = 0
    i = 0
    while i < len(subs):
        take = []
        while len(take) < 2 and i < len(subs):
            take.append(subs[i])
            i += 1
            if take[-1] != 512:
                break  # non-512 sub must be last in its group (bank align)
        gw = sum(take)
        soff, ss = 0, []
        for w in take:
            ss.append((soff, w))
            soff += w
        groups.append((off, gw, ss))
        off += gw
    return groups


# ----------------------------------------------------------------------------
# Program builder
# ----------------------------------------------------------------------------
def _build_program(sig):
    b_dev, half_sizes, half_jobs = sig
    _patch_act_tables()
    nc = bacc.Bacc("TRN2", target_bir_lowering=False, debug=False,
                   num_devices=NCORES)

    din = lambda n, s, d=R: nc.dram_tensor(n, list(s), d, kind="ExternalInput")
    dout = lambda n, s: nc.dram_tensor(n, list(s), DT, kind="ExternalOutput")

    # all weights host-prepacked to [128, kt, out] so every DMA is a plain
    # nested slice (rearranged/offset DRAM access patterns fault the DGE)
    xT = din("xT", (128, 8, b_dev))
    ydm = din("yd", (2, b_dev))
    tw0, tw1 = din("tw0", (128, 8, L)), din("tw1", (128, 2, L))
    tw2 = din("tw2", (128, 2, K))
    yw0, yw1 = din("yw0", (128, 8, H)), din("yw1", (128, 8, H))
    dw0, dw1 = din("dw0", (128, 8, H)), din("dw1", (128, 8, H))
    zw0, zw1 = din("zw0", (128, 8, H)), din("zw1", (128, 8, H))
    zw0yd = din("zw0yd", (2, 8, 128))
    yhw, dhw = din("yhw", (128, 8, 2 * K)), din("dhw", (128, 8, 2 * K))
    zhW = din("zhW", (K, 128, 8, 2 * L))
    # biases (host-prepacked layouts)
    tb0 = din("tb0", (128, 2), DT)
    tb1 = din("tb1", (128, 2), DT)
    tb2 = din("tb2", (K, 1), DT)
    yb0, yb1 = din("yb0", (128, 8), DT), din("yb1", (128, 8), DT)
    db0, db1 = din("db0", (128, 8), DT), din("db1", (128, 8), DT)
    zb0, zb1 = din("zb0", (128, 8), DT), din("zb1", (128, 8), DT)
    yhb, dhb = din("yhb", (2 * K, 1), DT), din("dhb", (2 * K, 1), DT)
    zhb = din("zhb", (128, 4 * K), DT)  # col = k*4 + mj

    tlog = dout("tlog", (K, b_dev))
    yall = dout("yall", (2 * K, b_dev))
    dall = dout("dall", (2 * K, b_dev))
    zloc = dout("zloc", (L, b_dev))
    zscale = dout("zscale", (L, b_dev))

    with tile.TileContext(nc) as tc:
        with (
            tc.tile_pool(name="const", bufs=1) as pc,
            tc.tile_pool(name="px", bufs=1) as px,
            tc.tile_pool(name="pact", bufs=1) as pact,
            tc.tile_pool(name="pw", bufs=5) as pw,
            tc.tile_pool(name="pwyd", bufs=2) as pwyd,
            tc.tile_pool(name="pe", bufs=3) as pe_,
            tc.tile_pool(name="po", bufs=3) as po,
            tc.tile_pool(name="pyd", bufs=1) as pyd,
            tc.tile_pool(name="pzs", bufs=1) as pzs,
            tc.tile_pool(name="pps", bufs=3, space="PSUM") as pps,
            tc.tile_pool(name="ppsh", bufs=2, space="PSUM") as ppsh,
        ):
            # ---- constants ----
            def cload(drm, shape, tag):
                t_ = pc.tile(list(shape), DT, tag=tag)
                nc.gpsimd.dma_start(t_[:], drm[:, :])
                return t_

            tb0t = cload(tb0, (128, 2), "tb0")
            tb1t = cload(tb1, (128, 2), "tb1")
            tb2t = cload(tb2, (K, 1), "tb2")
            yb0t = cload(yb0, (128, 8), "yb0")
            yb1t = cload(yb1, (128, 8), "yb1")
            db0t = cload(db0, (128, 8), "db0")
            db1t = cload(db1, (128, 8), "db1")
            zb0t = cload(zb0, (128, 8), "zb0")
            zb1t = cload(zb1, (128, 8), "zb1")
            yhbt = cload(yhb, (2 * K, 1), "yhb")
            dhbt = cload(dhb, (2 * K, 1), "dhb")
            zhbt = cload(zhb, (128, 4 * K), "zhb")
            # y/d head weights, resident (one merged DMA each)
            yhwt = pc.tile([128, 8, 2 * K], R, tag="yhwt")
            dhwt = pc.tile([128, 8, 2 * K], R, tag="dhwt")
            nc.gpsimd.dma_start(yhwt[:, :, :], yhw[:, :, :])
            nc.gpsimd.dma_start(dhwt[:, :, :], dhw[:, :, :])

            def elu_post(ps, gw, bias, dst, extra_min=None, part=128):
                # writes elu(x)+1 = relu(x) + min(exp(x), 1); downstream
                # biases are host-adjusted by -colsum(W) to compensate.
                e = pe_.tile([part, gw], DT, tag="e")
                nc.vector.tensor_scalar(dst, ps[:, :gw], bias, 0.0,
                                        ALU.add, ALU.max)
                nc.scalar.activation(e[:, :], ps[:, :gw], AF.Exp, bias=bias)
                nc.vector.scalar_tensor_tensor(dst, e[:, :], 1.0, dst,
                                               ALU.min, ALU.add)
                if extra_min is not None:  # final output: undo +1, clip
                    nc.vector.tensor_scalar(dst, dst, -1.0, extra_min,
                                            ALU.add, ALU.min)

            def emit_layer(src, kt, wdram, mt, bias_t, dst, groups,
                           extra=None):
                """dst(mi) -> AP [128, BH]; src(ki) -> AP [128, BH]."""
                for mi in range(mt):
                    wt = pw.tile([128, kt, 128], R, tag="W")
                    nc.sync.dma_start(
                        wt[:, :, :],
                        wdram[:, :, 128 * mi:128 * (mi + 1)])
                    ex = extra(mi) if extra is not None else None
                    for (goff, gw, ss) in groups:
                        ps = pps.tile([128, gw], DT, tag="acc")
                        for (soff, sw) in ss:
                            a = goff + soff
                            for ki in range(kt):
                                nc.tensor.matmul(
                                    ps[:, soff:soff + sw], wt[:, ki, :],
                                    src(ki)[:, a:a + sw],
                                    start=(ki == 0),
                                    stop=(ki == kt - 1 and ex is None))
                            if ex is not None:
                                lhs, rhs = ex
                                nc.tensor.matmul(ps[:, soff:soff + sw], lhs,
                                                 rhs[:, a:a + sw],
                                                 start=False, stop=True)
                        elu_post(ps, gw, bias_t[:, mi:mi + 1],
                                 dst(mi)[:, goff:goff + gw])

            def emit_head(src, kt, wtile, p_out, bias_col, out_dram, hoff,
                          groups, mode):
                """Small-head layer: p_out<=128 output features, one m-tile."""
                for (goff, gw, ss) in groups:
                    ps = pps.tile([p_out, gw], DT, tag="acc")
                    for (soff, sw) in ss:
                        a = goff + soff
                        for ki in range(kt):
                            nc.tensor.matmul(ps[:, soff:soff + sw],
                                             wtile(ki), src(ki)[:, a:a + sw],
                                             start=(ki == 0),
                                             stop=(ki == kt - 1))
                    ot = po.tile([p_out, gw], DT, tag="oS")
                    if mode == "eluclip":
                        elu_post(ps, gw, bias_col, ot[:, :], extra_min=10.0,
                                 part=p_out)
                    else:  # raw + bias
                        nc.scalar.activation(ot[:, :], ps[:, :gw], AF.Identity,
                                             bias=bias_col)
                    nc.gpsimd.dma_start(
                        out_dram[0:p_out, hoff + goff:hoff + goff + gw],
                        ot[:, :])

            hoff = 0
            for hf in (0, 1):
                bh = half_sizes[hf]
                groups = _chunk_groups(bh)
                xh = px.tile([128, 8, bh], R, tag="x")
                nc.sync.dma_start(xh[:, :, :], xT[:, :, hoff:hoff + bh])
                ydh = pyd.tile([2, bh], R, tag="yd")
                nc.sync.dma_start(ydh[:, :], ydm[:, hoff:hoff + bh])

                xsrc = lambda ki: xh[:, ki, :]

                # ---- z branch ----
                wydt = pwyd.tile([2, 8, 128], R, tag="wyd")
                nc.sync.dma_start(wydt[:, :, :], zw0yd[:, :, :])

                hz1 = pact.tile([128, 8, bh], R, tag="hA")
                emit_layer(xsrc, 8, zw0, 8, zb0t,
                           lambda mi: hz1[:, mi, :], groups,
                           extra=lambda mi: (wydt[:, mi, :], ydh))
                hz2 = pact.tile([128, 8, bh], R, tag="hB")
                emit_layer(lambda ki: hz1[:, ki, :], 8, zw1, 8, zb1t,
                           lambda mi: hz2[:, mi, :], groups)

                # ---- z head (routed) ----
                zs = pzs.tile([128, 2, bh], DT, tag="zs")  # raw scale staging
                for (k, loff, n) in half_jobs[hf]:
                    for mj in range(4):
                        wt = pw.tile([128, 8, 128], R, tag="W")
                        nc.sync.dma_start(
                            wt[:, :, :],
                            zhW[k, :, :, 128 * mj:128 * (mj + 1)])
                        ps = ppsh.tile([128, n], DT, tag="acch")
                        for ki in range(8):
                            nc.tensor.matmul(ps[:, :],
                                             wt[:, ki, :],
                                             hz2[:, ki, loff:loff + n],
                                             start=(ki == 0), stop=(ki == 7))
                        bias = zhbt[:, 4 * k + mj:4 * k + mj + 1]
                        if mj < 2:
                            ot = po.tile([128, n], DT, tag="oS")
                            nc.vector.tensor_scalar(ot[:, :], ps[:, :],
                                                    bias, -100.0,
                                                    ALU.add, ALU.max)
                            nc.vector.tensor_scalar_min(ot[:, :], ot[:, :],
                                                        100.0)
                            nc.gpsimd.dma_start(
                                zloc[128 * mj:128 * (mj + 1),
                                     hoff + loff:hoff + loff + n], ot[:, :])
                        else:
                            nc.vector.tensor_scalar(
                                zs[:, mj - 2, loff:loff + n], ps[:, :],
                                bias, None, ALU.add)
                # ---- t branch ----
                h1 = pact.tile([128, 2, bh], R, tag="hA")
                emit_layer(xsrc, 8, tw0, 2, tb0t,
                           lambda mi: h1[:, mi, :], groups)
                h2 = pact.tile([128, 2, bh], R, tag="hB")
                emit_layer(lambda ki: h1[:, ki, :], 2, tw1, 2, tb1t,
                           lambda mi: h2[:, mi, :], groups)
                tw2t = pwyd.tile([128, 2, K], R, tag="tw2")
                nc.sync.dma_start(tw2t[:, :, :], tw2[:, :, :])
                emit_head(lambda ki: h2[:, ki, :], 2,
                          lambda ki: tw2t[:, ki, :], K, tb2t[:, 0:1],
                          tlog, hoff, groups, "eluclip")

                # ---- y branch ----
                hy1 = pact.tile([128, 8, bh], R, tag="hA")
                emit_layer(xsrc, 8, yw0, 8, yb0t,
                           lambda mi: hy1[:, mi, :], groups)
                hy2 = pact.tile([128, 8, bh], R, tag="hB")
                emit_layer(lambda ki: hy1[:, ki, :], 8, yw1, 8, yb1t,
                           lambda mi: hy2[:, mi, :], groups)
                emit_head(lambda ki: hy2[:, ki, :], 8,
                          lambda ki: yhwt[:, ki, :], 2 * K, yhbt[:, 0:1],
                          yall, hoff, groups, "raw")

                # batched softplus over the half's raw scale staging:
                # softplus(x) = relu(x) + ln(1 + exp(-|x|)).  Ops are phase-
                # ordered (all Relu/Abs/Exp, then all Ln) so the ACT table
                # switches exp<->ln at most once per half.
                all_units = [(mj, g) for mj in range(2) for g in groups]
                unit_chunks = [all_units[i:i + 2]
                               for i in range(0, len(all_units), 2)]
                for units in unit_chunks:
                  rts, e2s = [], []
                  for (mj, (goff, gw, _)) in units:
                    zsl = zs[:, mj, goff:goff + gw]
                    ot = po.tile([128, gw], DT, tag="oS")
                    nc.scalar.activation(ot[:, :], zsl, AF.Relu)
                    rts.append(ot)
                  for (mj, (goff, gw, _)) in units:
                    zsl = zs[:, mj, goff:goff + gw]
                    e1 = pe_.tile([128, gw], DT, tag="e")
                    nc.scalar.activation(e1[:, :], zsl, AF.Abs)
                    e2 = pe_.tile([128, gw], DT, tag="e")
                    nc.scalar.activation(e2[:, :], e1[:, :], AF.Exp,
                                         scale=-1.0)
                    e2s.append(e2)
                  for u, (mj, (goff, gw, _)) in enumerate(units):
                    zsl = zs[:, mj, goff:goff + gw]
                    nc.scalar.activation(zsl, e2s[u][:, :], AF.Ln, bias=1.0)
                    ot = rts[u]
                    nc.vector.tensor_tensor(ot[:, :], ot[:, :], zsl, ALU.add)
                    nc.vector.tensor_scalar(ot[:, :], ot[:, :], 0.001, 100.0,
                                            ALU.add, ALU.min)
                    nc.gpsimd.dma_start(
                        zscale[128 * mj:128 * (mj + 1),
                               hoff + goff:hoff + goff + gw], ot[:, :])
                # ---- d branch ----
                hd1 = pact.tile([128, 8, bh], R, tag="hA")
                emit_layer(xsrc, 8, dw0, 8, db0t,
                           lambda mi: hd1[:, mi, :], groups)
                hd2 = pact.tile([128, 8, bh], R, tag="hB")
                emit_layer(lambda ki: hd1[:, ki, :], 8, dw1, 8, db1t,
                           lambda mi: hd2[:, mi, :], groups)
                emit_head(lambda ki: hd2[:, ki, :], 8,
                          lambda ki: dhwt[:, ki, :], 2 * K, dhbt[:, 0:1],
                          dall, hoff, groups, "raw")

                hoff += bh

    nc.compile()
    return nc


def _get_program(sig):
    if sig not in _PROG_CACHE:
        _PROG_CACHE[sig] = _build_program(sig)
    return _PROG_CACHE[sig]


# ----------------------------------------------------------------------------
# Host-side glue
# ----------------------------------------------------------------------------
def _softplus64(x):
    x = x.astype(np.float64)
    return np.log1p(np.exp(-np.abs(x))) + np.maximum(x, 0.0)


def _f32(a):
    return np.ascontiguousarray(np.asarray(a), dtype=np.float32)


def make_in_maps(inputs, plan):
    x = _f32(inputs["x"])
    y = _f32(inputs["y"])
    d = _f32(inputs["d"])
    perm = plan["perm"]

    def pack_bias(v, rows=128):
        v = _f32(v).reshape(-1)
        if len(v) % rows == 0 and len(v) >= rows:
            return np.ascontiguousarray(v.reshape(-1, rows).T)
        return v.reshape(-1, 1)

    # the device's hidden ELU outputs are shifted by +1 (fused op form);
    # compensate in every consumer's bias: b' = b - colsum(W)
    def csum(w):
        return np.asarray(w).astype(np.float64).sum(axis=0)

    def pack(w):
        """[kt*128, O] -> [128, kt, O] (partition-major weight layout)."""
        w = _f32(w)
        kt = w.shape[0] // 128
        return np.ascontiguousarray(
            w.reshape(kt, 128, w.shape[1]).transpose(1, 0, 2))

    yhw = _f32(np.transpose(np.asarray(inputs["yhW"]), (1, 0, 2))
               .reshape(H, 2 * K))
    dhw = _f32(np.transpose(np.asarray(inputs["dhW"]), (1, 0, 2))
               .reshape(H, 2 * K))
    zhW = _f32(inputs["zhW"])
    zhb_adj = (np.asarray(inputs["zhb"]).astype(np.float64)
               - zhW.astype(np.float64).sum(axis=1))  # [K, 512]
    zw0 = _f32(inputs["zw0"])
    shared = {
        "tw0": pack(inputs["tw0"]), "tw1": pack(inputs["tw1"]),
        "tw2": pack(inputs["tw2"]),
        "yw0": pack(inputs["yw0"]), "yw1": pack(inputs["yw1"]),
        "dw0": pack(inputs["dw0"]), "dw1": pack(inputs["dw1"]),
        "zw0": pack(zw0[2:]), "zw1": pack(inputs["zw1"]),
        "zw0yd": np.ascontiguousarray(zw0[:2].reshape(2, 8, 128)),
        "yhw": pack(yhw),
        "dhw": pack(dhw),
        "zhW": np.ascontiguousarray(
            zhW.reshape(K, 8, 128, 2 * L).transpose(0, 2, 1, 3)),
        "tb0": pack_bias(inputs["tb0"]),
        "tb1": pack_bias(np.asarray(inputs["tb1"]) - csum(inputs["tw1"])),
        "tb2": pack_bias(np.asarray(inputs["tb2"]) - csum(inputs["tw2"])),
        "yb0": pack_bias(inputs["yb0"]),
        "yb1": pack_bias(np.asarray(inputs["yb1"]) - csum(inputs["yw1"])),
        "db0": pack_bias(inputs["db0"]),
        "db1": pack_bias(np.asarray(inputs["db1"]) - csum(inputs["dw1"])),
        "zb0": pack_bias(inputs["zb0"]),
        "zb1": pack_bias(np.asarray(inputs["zb1"]) - csum(inputs["zw1"])),
        "yhb": _f32(np.asarray(inputs["yhb"]).reshape(2 * K)
                    - csum(yhw)).reshape(2 * K, 1),
        "dhb": _f32(np.asarray(inputs["dhb"]).reshape(2 * K)
                    - csum(dhw)).reshape(2 * K, 1),
        "zhb": np.ascontiguousarray(
            _f32(zhb_adj).reshape(K, 4, 128).transpose(2, 0, 1)
            .reshape(128, 4 * K)),
    }
    in_maps = []
    for c in range(NCORES):
        rows = perm[c]
        m = dict(shared)
        m["xT"] = np.ascontiguousarray(
            x[rows].T.reshape(8, 128, len(rows)).transpose(1, 0, 2))
        m["yd"] = np.ascontiguousarray(
            np.stack([y[rows], d[rows]], axis=0))
        in_maps.append(m)
    return in_maps


def assemble(results, plan, t):
    out = np.empty((B, K + 4 + 2 * L), dtype=np.float32)
    perm, valid = plan["perm"], plan["valid"]
    for c in range(NCORES):
        r = results[c]
        cols = np.nonzero(valid[c])[0]
        rows = perm[c][cols]
        out[rows, 0:K] = r["tlog"][:, cols].T
        tr = t[rows].astype(np.int64)
        ar = np.arange(len(cols))
        for name, o in (("yall", K), ("dall", K + 2)):
            a = r[name][:, cols]
            loc = np.clip(a[2 * tr, ar], -1e6, 1e6)
            scale = np.minimum(_softplus64(a[2 * tr + 1, ar]) + 1e-3, 1e6)
            out[rows, o] = loc
            out[rows, o + 1] = scale.astype(np.float32)
        out[rows, K + 4:K + 4 + L] = r["zloc"][:, cols].T
        out[rows, K + 4 + L:] = r["zscale"][:, cols].T
    return out


def kernel(**inputs) -> np.ndarray:
    t = np.asarray(inputs["t"]).astype(np.int32)
    plan = _plan_routing(t)
    sig = (plan["b_dev"], plan["half_sizes"], plan["half_jobs"])
    nc = _get_program(sig)
    in_maps = make_in_maps(inputs, plan)
    res = run_bass_kernel_spmd(nc, in_maps, list(range(NCORES)))
    return assemble(res.results, plan, t)


# revision 19
# speedup vs baseline: 1.7020x; 1.0488x over previous
"""Trainium2 Bass kernel for the CEVAE-guide multi-head MLP (moe_routing).

Strategy:
  - Pure data parallel: batch B=16384 split across 8 NeuronCores (2048 each).
  - Host-side MoE routing for the per-treatment heads: each core's columns
    are grouped by treatment id into fixed-size slots (same layout on every
    core -> one SPMD program with static offsets). The z head then computes
    only the selected expert per column (contiguous slices), 7x less work
    than the dense reference.
  - Activations kept feature-on-partition ([feat, batch]) so the matmul
    chain needs no transposes; host pre-transposes x.
  - Matmuls run as float32r (full PE rate for fp32 data, ~1e-4/layer err).
  - ELU = Relu(x) + (min(Exp(x),1) - 1), biases folded into the ACT pass.
  - y/d head treatment-selection (2 of 14 raw columns) + their tiny scalar
    nonlinearities run on host; all heavy math runs on device.
"""

import os
import sys

import numpy as np

sys.path.insert(0, "/opt/trn_rl_repo")

import concourse.bass as bass  # noqa: E402
import concourse.mybir as mybir  # noqa: E402
import concourse.tile as tile  # noqa: E402
from concourse import bacc  # noqa: E402
from concourse.bass_utils import run_bass_kernel_spmd  # noqa: E402

B, F, H, L, K = 16384, 1024, 1024, 256, 7
NCORES = 8
DT = mybir.dt.float32
R = mybir.dt.float32 if os.environ.get("CEVAE_FP32") else mybir.dt.float32r
AF = mybir.ActivationFunctionType
ALU = mybir.AluOpType

_PROG_CACHE: dict = {}

# Every ACT function this kernel uses (Relu, Exp, Abs, Ln, Identity) lives in
# the natural_log_exp_and_others table; restricting the table-load pass to it
# yields a single LoadActFuncSet instead of exp<->ln thrash (~2.7us each).
_ACT_KEEP = "natural_log_exp_and_others"


def _patch_act_tables():
    from concourse import bacc as _bacc_mod
    orig = _bacc_mod.get_activation_tables
    if getattr(orig, "_cevae_patched", False):
        return
    def patched(arch):
        tabs = orig(arch)
        if _ACT_KEEP in tabs:
            tabs = {name: (funcs if name == _ACT_KEEP else set())
                    for name, funcs in tabs.items()}
        return tabs
    patched._cevae_patched = True
    _bacc_mod.get_activation_tables = patched


# ----------------------------------------------------------------------------
# Routing plan
# ----------------------------------------------------------------------------
def _plan_routing(t: np.ndarray):
    """Group batch rows by treatment into per-core column slots.

    Every core gets the same slot layout (n_slots per expert job), so a
    single SPMD program with compile-time offsets serves all cores.
    """
    jobs = []  # (expert, n_slots, [per-core row-index arrays])
    for k in range(K):
        idx = np.nonzero(t == k)[0]
        need = max(1, -(-len(idx) // NCORES))
        nparts = max(1, -(-need // 512))  # keep each job's slots <= 512
        for part in np.array_split(idx, nparts):
            per_core = np.array_split(part, NCORES)
            n_slots = max(256, max(len(p) for p in per_core))
            n_slots = min((n_slots + 1) // 2 * 2, 512)  # even (fp32r ISA)
            jobs.append((k, n_slots, per_core))

    # balance jobs into two halves (expert-set split) by slot count
    order = sorted(range(len(jobs)), key=lambda j: -jobs[j][1])
    tot, sets = [0, 0], [[], []]
    for j in order:
        s = 0 if tot[0] <= tot[1] else 1
        sets[s].append(j)
        tot[s] += jobs[j][1]

    layout = []  # job ids in column order
    half_jobs = [[], []]  # (expert, local_off, n_slots)
    half_sizes = []
    for s in (0, 1):
        off = 0
        for j in sorted(sets[s]):
            k, n_slots, _ = jobs[j]
            half_jobs[s].append((k, off, n_slots))
            layout.append(j)
            off += n_slots
        half_sizes.append(off)
    b_dev = sum(half_sizes)

    perm = np.zeros((NCORES, b_dev), dtype=np.int64)
    valid = np.zeros((NCORES, b_dev), dtype=bool)
    for c in range(NCORES):
        off = 0
        for j in layout:
            _, n_slots, per_core = jobs[j]
            rows = per_core[c]
            n = len(rows)
            perm[c, off:off + n] = rows
            valid[c, off:off + n] = True
            filler = rows[0] if n else 0
            perm[c, off + n:off + n_slots] = filler
            off += n_slots

    return {
        "b_dev": b_dev,
        "half_sizes": tuple(half_sizes),
        "half_jobs": (tuple(half_jobs[0]), tuple(half_jobs[1])),
        "perm": perm,
        "valid": valid,
    }


def _chunk_groups(bh: int):
    """Split a half's `bh` columns into PSUM groups (<=1536 wide, <=3 banks)
    of matmul subchunks. Each matmul's output must stay inside one 2KB PSUM
    bank (512 fp32), so every non-final sub in a group is exactly 512;
    widths are even (fp32r ISA) and kept >=256 where possible (fp32r
    full-rate threshold)."""
    assert bh % 2 == 0
    q, r = divmod(bh, 512)
    if r == 0:
        subs = [512] * q
    elif r >= 256 or q == 0:
        subs = [512] * q + [r]
    else:  # split the last 512+r into two balanced subs in [256, 384]
        s1 = (512 + r) // 4 * 2
        subs = [512] * (q - 1) + [s1, 512 + r - s1]
    groups = []  # (group_off, group_width, [(sub_off, sub_w), ...])
    off = 0
    i = 0
    while i < len(subs):
        take = []
        while len(take) < 2 and i < len(subs):
            take.append(subs[i])
            i += 1
            if take[-1] != 512:
                break  # non-512 sub must be last in its group (bank align)
        gw = sum(take)
        soff, ss = 0, []
        for w in take:
            ss.append((soff, w))
            soff += w
        groups.append((off, gw, ss))
        off += gw
    return groups


# ----------------------------------------------------------------------------
# Program builder
# ----------------------------------------------------------------------------
def _build_program(sig):
    b_dev, half_sizes, half_jobs = sig
    _patch_act_tables()
    nc = bacc.Bacc("TRN2", target_bir_lowering=False, debug=False,
                   num_devices=NCORES)

    din = lambda n, s, d=R: nc.dram_tensor(n, list(s), d, kind="ExternalInput")
    dout = lambda n, s: nc.dram_tensor(n, list(s), DT, kind="ExternalOutput")

    # all weights host-prepacked to [128, kt, out] so every DMA is a plain
    # nested slice (rearranged/offset DRAM access patterns fault the DGE)
    xT = din("xT", (128, 8, b_dev))
    ydm = din("yd", (2, b_dev))
    tw0, tw1 = din("tw0", (128, 8, L)), din("tw1", (128, 2, L))
    tw2 = din("tw2", (128, 2, K))
    yw0, yw1 = din("yw0", (128, 8, H)), din("yw1", (128, 8, H))
    dw0, dw1 = din("dw0", (128, 8, H)), din("dw1", (128, 8, H))
    zw0, zw1 = din("zw0", (128, 8, H)), din("zw1", (128, 8, H))
    zw0yd = din("zw0yd", (2, 8, 128))
    yhw, dhw = din("yhw", (128, 8, 2 * K)), din("dhw", (128, 8, 2 * K))
    zhW = din("zhW", (K, 128, 8, 2 * L))
    # biases (host-prepacked layouts)
    tb0 = din("tb0", (128, 2), DT)
    tb1 = din("tb1", (128, 2), DT)
    tb2 = din("tb2", (K, 1), DT)
    yb0, yb1 = din("yb0", (128, 8), DT), din("yb1", (128, 8), DT)
    db0, db1 = din("db0", (128, 8), DT), din("db1", (128, 8), DT)
    zb0, zb1 = din("zb0", (128, 8), DT), din("zb1", (128, 8), DT)
    yhb, dhb = din("yhb", (2 * K, 1), DT), din("dhb", (2 * K, 1), DT)
    zhb = din("zhb", (128, 4 * K), DT)  # col = k*4 + mj

    tlog = dout("tlog", (K, b_dev))
    yall = dout("yall", (2 * K, b_dev))
    dall = dout("dall", (2 * K, b_dev))
    zloc = dout("zloc", (L, b_dev))
    zscale = dout("zscale", (L, b_dev))

    with tile.TileContext(nc) as tc:
        with (
            tc.tile_pool(name="const", bufs=1) as pc,
            tc.tile_pool(name="px", bufs=1) as px,
            tc.tile_pool(name="pact", bufs=1) as pact,
            tc.tile_pool(name="pw", bufs=5) as pw,
            tc.tile_pool(name="pwyd", bufs=2) as pwyd,
            tc.tile_pool(name="pe", bufs=3) as pe_,
            tc.tile_pool(name="po", bufs=3) as po,
            tc.tile_pool(name="pyd", bufs=1) as pyd,
            tc.tile_pool(name="pzs", bufs=1) as pzs,
            tc.tile_pool(name="pps", bufs=4, space="PSUM") as pps,
        ):
            # ---- constants ----
            def cload(drm, shape, tag):
                t_ = pc.tile(list(shape), DT, tag=tag)
                nc.gpsimd.dma_start(t_[:], drm[:, :])
                return t_

            tb0t = cload(tb0, (128, 2), "tb0")
            tb1t = cload(tb1, (128, 2), "tb1")
            tb2t = cload(tb2, (K, 1), "tb2")
            yb0t = cload(yb0, (128, 8), "yb0")
            yb1t = cload(yb1, (128, 8), "yb1")
            db0t = cload(db0, (128, 8), "db0")
            db1t = cload(db1, (128, 8), "db1")
            zb0t = cload(zb0, (128, 8), "zb0")
            zb1t = cload(zb1, (128, 8), "zb1")
            yhbt = cload(yhb, (2 * K, 1), "yhb")
            dhbt = cload(dhb, (2 * K, 1), "dhb")
            zhbt = cload(zhb, (128, 4 * K), "zhb")
            # y/d head weights, resident (one merged DMA each)
            yhwt = pc.tile([128, 8, 2 * K], R, tag="yhwt")
            dhwt = pc.tile([128, 8, 2 * K], R, tag="dhwt")
            nc.gpsimd.dma_start(yhwt[:, :, :], yhw[:, :, :])
            nc.gpsimd.dma_start(dhwt[:, :, :], dhw[:, :, :])

            def elu_post(ps, gw, bias, dst, extra_min=None, part=128):
                # writes elu(x)+1 = relu(x) + min(exp(x), 1); downstream
                # biases are host-adjusted by -colsum(W) to compensate.
                e = pe_.tile([part, gw], DT, tag="e")
                nc.vector.tensor_scalar(dst, ps[:, :gw], bias, 0.0,
                                        ALU.add, ALU.max)
                nc.scalar.activation(e[:, :], ps[:, :gw], AF.Exp, bias=bias)
                nc.vector.scalar_tensor_tensor(dst, e[:, :], 1.0, dst,
                                               ALU.min, ALU.add)
                if extra_min is not None:  # final output: undo +1, clip
                    nc.vector.tensor_scalar(dst, dst, -1.0, extra_min,
                                            ALU.add, ALU.min)

            def emit_layer(src, kt, wdram, mt, bias_t, dst, groups,
                           extra=None, pre_w=None):
                """dst(mi) -> AP [128, BH]; src(ki) -> AP [128, BH]."""
                for mi in range(mt):
                    if mi == 0 and pre_w is not None:
                        wt = pre_w
                    else:
                        wt = pw.tile([128, kt, 128], R, tag="W")
                        nc.sync.dma_start(
                            wt[:, :, :],
                            wdram[:, :, 128 * mi:128 * (mi + 1)])
                    ex = extra(mi) if extra is not None else None
                    for (goff, gw, ss) in groups:
                        ps = pps.tile([128, gw], DT, tag="acc")
                        for (soff, sw) in ss:
                            a = goff + soff
                            if ex is not None:
                                lhs, rhs = ex
                                nc.tensor.matmul(ps[:, soff:soff + sw], lhs,
                                                 rhs[:, a:a + sw],
                                                 start=True, stop=False)
                            for ki in range(kt):
                                nc.tensor.matmul(
                                    ps[:, soff:soff + sw], wt[:, ki, :],
                                    src(ki)[:, a:a + sw],
                                    start=(ki == 0 and ex is None),
                                    stop=(ki == kt - 1))
                        elu_post(ps, gw, bias_t[:, mi:mi + 1],
                                 dst(mi)[:, goff:goff + gw])

            def emit_head(src, kt, wtile, p_out, bias_col, out_dram, hoff,
                          groups, mode):
                """Small-head layer: p_out<=128 output features, one m-tile."""
                for (goff, gw, ss) in groups:
                    ps = pps.tile([p_out, gw], DT, tag="acc")
                    for (soff, sw) in ss:
                        a = goff + soff
                        for ki in range(kt):
                            nc.tensor.matmul(ps[:, soff:soff + sw],
                                             wtile(ki), src(ki)[:, a:a + sw],
                                             start=(ki == 0),
                                             stop=(ki == kt - 1))
                    ot = po.tile([p_out, gw], DT, tag="oS")
                    if mode == "eluclip":
                        elu_post(ps, gw, bias_col, ot[:, :], extra_min=10.0,
                                 part=p_out)
                    else:  # raw + bias
                        nc.scalar.activation(ot[:, :], ps[:, :gw], AF.Identity,
                                             bias=bias_col)
                    nc.gpsimd.dma_start(
                        out_dram[0:p_out, hoff + goff:hoff + goff + gw],
                        ot[:, :])

            hoff = 0
            for hf in (0, 1):
                bh = half_sizes[hf]
                groups = _chunk_groups(bh)
                ydh = pyd.tile([2, bh], R, tag="yd")
                nc.sync.dma_start(ydh[:, :], ydm[:, hoff:hoff + bh])
                wydt = pwyd.tile([2, 8, 128], R, tag="wyd")
                nc.sync.dma_start(wydt[:, :, :], zw0yd[:, :, :])
                zw0m0 = pw.tile([128, 8, 128], R, tag="W")
                nc.sync.dma_start(zw0m0[:, :, :], zw0[:, :, 0:128])
                xks = []
                for ki in range(8):
                    xk = px.tile([128, bh], R, tag=f"x{ki}")
                    nc.sync.dma_start(xk[:, :],
                                      xT[:, ki, hoff:hoff + bh])
                    xks.append(xk)

                xsrc = lambda ki: xks[ki][:, :]

                # ---- z branch ----
                hz1 = pact.tile([128, 8, bh], R, tag="hA")
                emit_layer(xsrc, 8, zw0, 8, zb0t,
                           lambda mi: hz1[:, mi, :], groups,
                           extra=lambda mi: (wydt[:, mi, :], ydh),
                           pre_w=zw0m0)
                hz2 = pact.tile([128, 8, bh], R, tag="hB")
                emit_layer(lambda ki: hz1[:, ki, :], 8, zw1, 8, zb1t,
                           lambda mi: hz2[:, mi, :], groups)

                # ---- z head (routed) ----
                zs = pzs.tile([128, 2, bh], DT, tag="zs")  # raw scale staging
                for (k, loff, n) in half_jobs[hf]:
                    for mj in range(4):
                        wt = pw.tile([128, 8, 128], R, tag="W")
                        nc.sync.dma_start(
                            wt[:, :, :],
                            zhW[k, :, :, 128 * mj:128 * (mj + 1)])
                        ps = pps.tile([128, n], DT, tag="acc")
                        for ki in range(8):
                            nc.tensor.matmul(ps[:, :],
                                             wt[:, ki, :],
                                             hz2[:, ki, loff:loff + n],
                                             start=(ki == 0), stop=(ki == 7))
                        bias = zhbt[:, 4 * k + mj:4 * k + mj + 1]
                        if mj < 2:
                            ot = po.tile([128, n], DT, tag="oS")
                            nc.vector.tensor_scalar(ot[:, :], ps[:, :],
                                                    bias, -100.0,
                                                    ALU.add, ALU.max)
                            nc.vector.tensor_scalar_min(ot[:, :], ot[:, :],
                                                        100.0)
                            nc.gpsimd.dma_start(
                                zloc[128 * mj:128 * (mj + 1),
                                     hoff + loff:hoff + loff + n], ot[:, :])
                        else:
                            nc.vector.tensor_scalar(
                                zs[:, mj - 2, loff:loff + n], ps[:, :],
                                bias, None, ALU.add)
                # ---- t branch ----
                h1 = pact.tile([128, 2, bh], R, tag="hA")
                emit_layer(xsrc, 8, tw0, 2, tb0t,
                           lambda mi: h1[:, mi, :], groups)
                h2 = pact.tile([128, 2, bh], R, tag="hB")
                emit_layer(lambda ki: h1[:, ki, :], 2, tw1, 2, tb1t,
                           lambda mi: h2[:, mi, :], groups)
                tw2t = pwyd.tile([128, 2, K], R, tag="tw2")
                nc.sync.dma_start(tw2t[:, :, :], tw2[:, :, :])
                emit_head(lambda ki: h2[:, ki, :], 2,
                          lambda ki: tw2t[:, ki, :], K, tb2t[:, 0:1],
                          tlog, hoff, groups, "eluclip")

                # ---- y branch ----
                hy1 = pact.tile([128, 8, bh], R, tag="hA")
                emit_layer(xsrc, 8, yw0, 8, yb0t,
                           lambda mi: hy1[:, mi, :], groups)
                hy2 = pact.tile([128, 8, bh], R, tag="hB")
                emit_layer(lambda ki: hy1[:, ki, :], 8, yw1, 8, yb1t,
                           lambda mi: hy2[:, mi, :], groups)
                emit_head(lambda ki: hy2[:, ki, :], 8,
                          lambda ki: yhwt[:, ki, :], 2 * K, yhbt[:, 0:1],
                          yall, hoff, groups, "raw")

                # batched softplus over the half's raw scale staging:
                # softplus(x) = relu(x) + ln(1 + exp(-|x|)).  Ops are phase-
                # ordered (all Relu/Abs/Exp, then all Ln) so the ACT table
                # switches exp<->ln at most once per half.
                all_units = [(mj, g) for mj in range(2) for g in groups]
                unit_chunks = [all_units[i:i + 2]
                               for i in range(0, len(all_units), 2)]
                for units in unit_chunks:
                  rts, e2s = [], []
                  for (mj, (goff, gw, _)) in units:
                    zsl = zs[:, mj, goff:goff + gw]
                    ot = po.tile([128, gw], DT, tag="oS")
                    nc.scalar.activation(ot[:, :], zsl, AF.Relu)
                    rts.append(ot)
                  for (mj, (goff, gw, _)) in units:
                    zsl = zs[:, mj, goff:goff + gw]
                    e1 = pe_.tile([128, gw], DT, tag="e")
                    nc.scalar.activation(e1[:, :], zsl, AF.Abs)
                    e2 = pe_.tile([128, gw], DT, tag="e")
                    nc.scalar.activation(e2[:, :], e1[:, :], AF.Exp,
                                         scale=-1.0)
                    e2s.append(e2)
                  for u, (mj, (goff, gw, _)) in enumerate(units):
                    zsl = zs[:, mj, goff:goff + gw]
                    nc.scalar.activation(zsl, e2s[u][:, :], AF.Ln, bias=1.0)
                    ot = rts[u]
                    nc.vector.tensor_tensor(ot[:, :], ot[:, :], zsl, ALU.add)
                    nc.vector.tensor_scalar(ot[:, :], ot[:, :], 0.001, 100.0,
                                            ALU.add, ALU.min)
                    nc.gpsimd.dma_start(
                        zscale[128 * mj:128 * (mj + 1),
                               hoff + goff:hoff + goff + gw], ot[:, :])
                # ---- d branch ----
                hd1 = pact.tile([128, 8, bh], R, tag="hA")
                emit_layer(xsrc, 8, dw0, 8, db0t,
                           lambda mi: hd1[:, mi, :], groups)
                hd2 = pact.tile([128, 8, bh], R, tag="hB")
                emit_layer(lambda ki: hd1[:, ki, :], 8, dw1, 8, db1t,
                           lambda mi: hd2[:, mi, :], groups)
                emit_head(lambda ki: hd2[:, ki, :], 8,
                          lambda ki: dhwt[:, ki, :], 2 * K, dhbt[:, 0:1],
                          dall, hoff, groups, "raw")

                hoff += bh

    nc.compile()
    return nc


def _get_program(sig):
    if sig not in _PROG_CACHE:
        _PROG_CACHE[sig] = _build_program(sig)
    return _PROG_CACHE[sig]


# ----------------------------------------------------------------------------
# Host-side glue
# ----------------------------------------------------------------------------
def _softplus64(x):
    x = x.astype(np.float64)
    return np.log1p(np.exp(-np.abs(x))) + np.maximum(x, 0.0)


def _f32(a):
    return np.ascontiguousarray(np.asarray(a), dtype=np.float32)


def make_in_maps(inputs, plan):
    x = _f32(inputs["x"])
    y = _f32(inputs["y"])
    d = _f32(inputs["d"])
    perm = plan["perm"]

    def pack_bias(v, rows=128):
        v = _f32(v).reshape(-1)
        if len(v) % rows == 0 and len(v) >= rows:
            return np.ascontiguousarray(v.reshape(-1, rows).T)
        return v.reshape(-1, 1)

    # the device's hidden ELU outputs are shifted by +1 (fused op form);
    # compensate in every consumer's bias: b' = b - colsum(W)
    def csum(w):
        return np.asarray(w).astype(np.float64).sum(axis=0)

    def pack(w):
        """[kt*128, O] -> [128, kt, O] (partition-major weight layout)."""
        w = _f32(w)
        kt = w.shape[0] // 128
        return np.ascontiguousarray(
            w.reshape(kt, 128, w.shape[1]).transpose(1, 0, 2))

    yhw = _f32(np.transpose(np.asarray(inputs["yhW"]), (1, 0, 2))
               .reshape(H, 2 * K))
    dhw = _f32(np.transpose(np.asarray(inputs["dhW"]), (1, 0, 2))
               .reshape(H, 2 * K))
    zhW = _f32(inputs["zhW"])
    zhb_adj = (np.asarray(inputs["zhb"]).astype(np.float64)
               - zhW.astype(np.float64).sum(axis=1))  # [K, 512]
    zw0 = _f32(inputs["zw0"])
    shared = {
        "tw0": pack(inputs["tw0"]), "tw1": pack(inputs["tw1"]),
        "tw2": pack(inputs["tw2"]),
        "yw0": pack(inputs["yw0"]), "yw1": pack(inputs["yw1"]),
        "dw0": pack(inputs["dw0"]), "dw1": pack(inputs["dw1"]),
        "zw0": pack(zw0[2:]), "zw1": pack(inputs["zw1"]),
        "zw0yd": np.ascontiguousarray(zw0[:2].reshape(2, 8, 128)),
        "yhw": pack(yhw),
        "dhw": pack(dhw),
        "zhW": np.ascontiguousarray(
            zhW.reshape(K, 8, 128, 2 * L).transpose(0, 2, 1, 3)),
        "tb0": pack_bias(inputs["tb0"]),
        "tb1": pack_bias(np.asarray(inputs["tb1"]) - csum(inputs["tw1"])),
        "tb2": pack_bias(np.asarray(inputs["tb2"]) - csum(inputs["tw2"])),
        "yb0": pack_bias(inputs["yb0"]),
        "yb1": pack_bias(np.asarray(inputs["yb1"]) - csum(inputs["yw1"])),
        "db0": pack_bias(inputs["db0"]),
        "db1": pack_bias(np.asarray(inputs["db1"]) - csum(inputs["dw1"])),
        "zb0": pack_bias(inputs["zb0"]),
        "zb1": pack_bias(np.asarray(inputs["zb1"]) - csum(inputs["zw1"])),
        "yhb": _f32(np.asarray(inputs["yhb"]).reshape(2 * K)
                    - csum(yhw)).reshape(2 * K, 1),
        "dhb": _f32(np.asarray(inputs["dhb"]).reshape(2 * K)
                    - csum(dhw)).reshape(2 * K, 1),
        "zhb": np.ascontiguousarray(
            _f32(zhb_adj).reshape(K, 4, 128).transpose(2, 0, 1)
            .reshape(128, 4 * K)),
    }
    in_maps = []
    for c in range(NCORES):
        rows = perm[c]
        m = dict(shared)
        m["xT"] = np.ascontiguousarray(
            x[rows].T.reshape(8, 128, len(rows)).transpose(1, 0, 2))
        m["yd"] = np.ascontiguousarray(
            np.stack([y[rows], d[rows]], axis=0))
        in_maps.append(m)
    return in_maps


def assemble(results, plan, t):
    out = np.empty((B, K + 4 + 2 * L), dtype=np.float32)
    perm, valid = plan["perm"], plan["valid"]
    for c in range(NCORES):
        r = results[c]
        cols = np.nonzero(valid[c])[0]
        rows = perm[c][cols]
        out[rows, 0:K] = r["tlog"][:, cols].T
        tr = t[rows].astype(np.int64)
        ar = np.arange(len(cols))
        for name, o in (("yall", K), ("dall", K + 2)):
            a = r[name][:, cols]
            loc = np.clip(a[2 * tr, ar], -1e6, 1e6)
            scale = np.minimum(_softplus64(a[2 * tr + 1, ar]) + 1e-3, 1e6)
            out[rows, o] = loc
            out[rows, o + 1] = scale.astype(np.float32)
        out[rows, K + 4:K + 4 + L] = r["zloc"][:, cols].T
        out[rows, K + 4 + L:] = r["zscale"][:, cols].T
    return out


def kernel(**inputs) -> np.ndarray:
    t = np.asarray(inputs["t"]).astype(np.int32)
    plan = _plan_routing(t)
    sig = (plan["b_dev"], plan["half_sizes"], plan["half_jobs"])
    nc = _get_program(sig)
    in_maps = make_in_maps(inputs, plan)
    res = run_bass_kernel_spmd(nc, in_maps, list(range(NCORES)))
    return assemble(res.results, plan, t)


# revision 22
# speedup vs baseline: 1.7264x; 1.0144x over previous
"""Trainium2 Bass kernel for the CEVAE-guide multi-head MLP (moe_routing).

Strategy:
  - Pure data parallel: batch B=16384 split across 8 NeuronCores (2048 each).
  - Host-side MoE routing for the per-treatment heads: each core's columns
    are grouped by treatment id into fixed-size slots (same layout on every
    core -> one SPMD program with static offsets). The z head then computes
    only the selected expert per column (contiguous slices), 7x less work
    than the dense reference.
  - Activations kept feature-on-partition ([feat, batch]) so the matmul
    chain needs no transposes; host pre-transposes x.
  - Matmuls run as float32r (full PE rate for fp32 data, ~1e-4/layer err).
  - ELU = Relu(x) + (min(Exp(x),1) - 1), biases folded into the ACT pass.
  - y/d head treatment-selection (2 of 14 raw columns) + their tiny scalar
    nonlinearities run on host; all heavy math runs on device.
"""

import os
import sys

import numpy as np

sys.path.insert(0, "/opt/trn_rl_repo")

import concourse.bass as bass  # noqa: E402
import concourse.mybir as mybir  # noqa: E402
import concourse.tile as tile  # noqa: E402
from concourse import bacc  # noqa: E402
from concourse.bass_utils import run_bass_kernel_spmd  # noqa: E402

B, F, H, L, K = 16384, 1024, 1024, 256, 7
NCORES = 8
DT = mybir.dt.float32
R = mybir.dt.float32 if os.environ.get("CEVAE_FP32") else mybir.dt.float32r
AF = mybir.ActivationFunctionType
ALU = mybir.AluOpType

_PROG_CACHE: dict = {}

# Every ACT function this kernel uses (Relu, Exp, Abs, Ln, Identity) lives in
# the natural_log_exp_and_others table; restricting the table-load pass to it
# yields a single LoadActFuncSet instead of exp<->ln thrash (~2.7us each).
_ACT_KEEP = "natural_log_exp_and_others"


def _patch_act_tables():
    from concourse import bacc as _bacc_mod
    orig = _bacc_mod.get_activation_tables
    if getattr(orig, "_cevae_patched", False):
        return
    def patched(arch):
        tabs = orig(arch)
        if _ACT_KEEP in tabs:
            tabs = {name: (funcs if name == _ACT_KEEP else set())
                    for name, funcs in tabs.items()}
        return tabs
    patched._cevae_patched = True
    _bacc_mod.get_activation_tables = patched


# ----------------------------------------------------------------------------
# Routing plan
# ----------------------------------------------------------------------------
def _plan_routing(t: np.ndarray):
    """Group batch rows by treatment into per-core column slots.

    Every core gets the same slot layout (n_slots per expert job), so a
    single SPMD program with compile-time offsets serves all cores.
    """
    jobs = []  # (expert, n_slots, [per-core row-index arrays])
    for k in range(K):
        idx = np.nonzero(t == k)[0]
        need = max(1, -(-len(idx) // NCORES))
        nparts = max(1, -(-need // 512))  # keep each job's slots <= 512
        for part in np.array_split(idx, nparts):
            per_core = np.array_split(part, NCORES)
            n_slots = max(256, max(len(p) for p in per_core))
            n_slots = min((n_slots + 1) // 2 * 2, 512)  # even (fp32r ISA)
            jobs.append((k, n_slots, per_core))

    # balance jobs into two halves (expert-set split) by slot count
    order = sorted(range(len(jobs)), key=lambda j: -jobs[j][1])
    tot, sets = [0, 0], [[], []]
    for j in order:
        s = 0 if tot[0] <= tot[1] else 1
        sets[s].append(j)
        tot[s] += jobs[j][1]

    layout = []  # job ids in column order
    half_jobs = [[], []]  # (expert, local_off, n_slots)
    half_sizes = []
    for s in (0, 1):
        off = 0
        for j in sorted(sets[s]):
            k, n_slots, _ = jobs[j]
            half_jobs[s].append((k, off, n_slots))
            layout.append(j)
            off += n_slots
        half_sizes.append(off)
    b_dev = sum(half_sizes)

    perm = np.zeros((NCORES, b_dev), dtype=np.int64)
    valid = np.zeros((NCORES, b_dev), dtype=bool)
    for c in range(NCORES):
        off = 0
        for j in layout:
            _, n_slots, per_core = jobs[j]
            rows = per_core[c]
            n = len(rows)
            perm[c, off:off + n] = rows
            valid[c, off:off + n] = True
            filler = rows[0] if n else 0
            perm[c, off + n:off + n_slots] = filler
            off += n_slots

    return {
        "b_dev": b_dev,
        "half_sizes": tuple(half_sizes),
        "half_jobs": (tuple(half_jobs[0]), tuple(half_jobs[1])),
        "perm": perm,
        "valid": valid,
    }


def _chunk_groups(bh: int):
    """Split a half's `bh` columns into PSUM groups (<=1536 wide, <=3 banks)
    of matmul subchunks. Each matmul's output must stay inside one 2KB PSUM
    bank (512 fp32), so every non-final sub in a group is exactly 512;
    widths are even (fp32r ISA) and kept >=256 where possible (fp32r
    full-rate threshold)."""
    assert bh % 2 == 0
    q, r = divmod(bh, 512)
    if r == 0:
        subs = [512] * q
    elif r >= 256 or q == 0:
        subs = [512] * q + [r]
    else:  # split the last 512+r into two balanced subs in [256, 384]
        s1 = (512 + r) // 4 * 2
        subs = [512] * (q - 1) + [s1, 512 + r - s1]
    groups = []  # (group_off, group_width, [(sub_off, sub_w), ...])
    off = 0
    i = 0
    while i < len(subs):
        take = []
        while len(take) < 2 and i < len(subs):
            take.append(subs[i])
            i += 1
            if take[-1] != 512:
                break  # non-512 sub must be last in its group (bank align)
        gw = sum(take)
        soff, ss = 0, []
        for w in take:
            ss.append((soff, w))
            soff += w
        groups.append((off, gw, ss))
        off += gw
    return groups


# ----------------------------------------------------------------------------
# Program builder
# ----------------------------------------------------------------------------
def _build_program(sig, passes=1):
    b_dev, half_sizes, half_jobs = sig
    _patch_act_tables()
    nc = bacc.Bacc("TRN2", target_bir_lowering=False, debug=False,
                   num_devices=NCORES)

    din = lambda n, s, d=R: nc.dram_tensor(n, list(s), d, kind="ExternalInput")
    dout = lambda n, s: nc.dram_tensor(n, list(s), DT, kind="ExternalOutput")

    # all weights host-prepacked to [128, kt, out] so every DMA is a plain
    # nested slice (rearranged/offset DRAM access patterns fault the DGE)
    xT = din("xT", (128, 8, b_dev))
    ydm = din("yd", (2, b_dev))
    tw0, tw1 = din("tw0", (128, 8, L)), din("tw1", (128, 2, L))
    tw2 = din("tw2", (128, 2, K))
    yw0, yw1 = din("yw0", (128, 8, H)), din("yw1", (128, 8, H))
    dw0, dw1 = din("dw0", (128, 8, H)), din("dw1", (128, 8, H))
    zw0, zw1 = din("zw0", (128, 8, H)), din("zw1", (128, 8, H))
    zw0yd = din("zw0yd", (2, 8, 128))
    yhw, dhw = din("yhw", (128, 8, 2 * K)), din("dhw", (128, 8, 2 * K))
    zhW = din("zhW", (K, 128, 8, 2 * L))
    # biases (host-prepacked layouts)
    tb0 = din("tb0", (128, 2), DT)
    tb1 = din("tb1", (128, 2), DT)
    tb2 = din("tb2", (K, 1), DT)
    yb0, yb1 = din("yb0", (128, 8), DT), din("yb1", (128, 8), DT)
    db0, db1 = din("db0", (128, 8), DT), din("db1", (128, 8), DT)
    zb0, zb1 = din("zb0", (128, 8), DT), din("zb1", (128, 8), DT)
    yhb, dhb = din("yhb", (2 * K, 1), DT), din("dhb", (2 * K, 1), DT)
    zhb = din("zhb", (128, 4 * K), DT)  # col = k*4 + mj

    tlog = dout("tlog", (K, b_dev))
    yall = dout("yall", (2 * K, b_dev))
    dall = dout("dall", (2 * K, b_dev))
    zloc = dout("zloc", (L, b_dev))
    zscale = dout("zscale", (L, b_dev))

    with tile.TileContext(nc) as tc:
        with (
            tc.tile_pool(name="const", bufs=1) as pc,
            tc.tile_pool(name="px", bufs=1) as px,
            tc.tile_pool(name="pact", bufs=1) as pact,
            tc.tile_pool(name="pw", bufs=7) as pw,
            tc.tile_pool(name="pwyd", bufs=3) as pwyd,
            tc.tile_pool(name="pe", bufs=5) as pe_,
            tc.tile_pool(name="po", bufs=5) as po,
            tc.tile_pool(name="pyd", bufs=1) as pyd,
            tc.tile_pool(name="pzs", bufs=1) as pzs,
            tc.tile_pool(name="pps", bufs=4, space="PSUM") as pps,
        ):
            # ---- constants ----
            def cload(drm, shape, tag):
                t_ = pc.tile(list(shape), DT, tag=tag)
                nc.gpsimd.dma_start(t_[:], drm[:, :])
                return t_

            tb0t = cload(tb0, (128, 2), "tb0")
            tb1t = cload(tb1, (128, 2), "tb1")
            tb2t = cload(tb2, (K, 1), "tb2")
            yb0t = cload(yb0, (128, 8), "yb0")
            yb1t = cload(yb1, (128, 8), "yb1")
            db0t = cload(db0, (128, 8), "db0")
            db1t = cload(db1, (128, 8), "db1")
            zb0t = cload(zb0, (128, 8), "zb0")
            zb1t = cload(zb1, (128, 8), "zb1")
            yhbt = cload(yhb, (2 * K, 1), "yhb")
            dhbt = cload(dhb, (2 * K, 1), "dhb")
            zhbt = cload(zhb, (128, 4 * K), "zhb")
            # y/d head weights, resident (one merged DMA each)
            yhwt = pc.tile([128, 8, 2 * K], R, tag="yhwt")
            dhwt = pc.tile([128, 8, 2 * K], R, tag="dhwt")
            nc.gpsimd.dma_start(yhwt[:, :, :], yhw[:, :, :])
            nc.gpsimd.dma_start(dhwt[:, :, :], dhw[:, :, :])

            def elu_post(ps, gw, bias, dst, extra_min=None, part=128):
                # writes elu(x)+1 = relu(x) + min(exp(x), 1); downstream
                # biases are host-adjusted by -colsum(W) to compensate.
                e = pe_.tile([part, gw], DT, tag="e")
                nc.vector.tensor_scalar(dst, ps[:, :gw], bias, 0.0,
                                        ALU.add, ALU.max)
                nc.scalar.activation(e[:, :], ps[:, :gw], AF.Exp, bias=bias)
                nc.vector.scalar_tensor_tensor(dst, e[:, :], 1.0, dst,
                                               ALU.min, ALU.add)
                if extra_min is not None:  # final output: undo +1, clip
                    nc.vector.tensor_scalar(dst, dst, -1.0, extra_min,
                                            ALU.add, ALU.min)

            def emit_layer(src, kt, wdram, mt, bias_t, dst, groups,
                           extra=None, pre_w=None):
                """dst(mi) -> AP [128, BH]; src(ki) -> AP [128, BH]."""
                for mi in range(mt):
                    if mi == 0 and pre_w is not None:
                        wt = pre_w
                    else:
                        wt = pw.tile([128, kt, 128], R, tag="W")
                        nc.sync.dma_start(
                            wt[:, :, :],
                            wdram[:, :, 128 * mi:128 * (mi + 1)])
                    ex = extra(mi) if extra is not None else None
                    for (goff, gw, ss) in groups:
                        ps = pps.tile([128, gw], DT, tag="acc")
                        for (soff, sw) in ss:
                            a = goff + soff
                            if ex is not None:
                                lhs, rhs = ex
                                nc.tensor.matmul(ps[:, soff:soff + sw], lhs,
                                                 rhs[:, a:a + sw],
                                                 start=True, stop=False)
                            for ki in range(kt):
                                nc.tensor.matmul(
                                    ps[:, soff:soff + sw], wt[:, ki, :],
                                    src(ki)[:, a:a + sw],
                                    start=(ki == 0 and ex is None),
                                    stop=(ki == kt - 1))
                        elu_post(ps, gw, bias_t[:, mi:mi + 1],
                                 dst(mi)[:, goff:goff + gw])

            def emit_head(src, kt, wtile, p_out, bias_col, out_dram, hoff,
                          groups, mode):
                """Small-head layer: p_out<=128 output features, one m-tile."""
                for (goff, gw, ss) in groups:
                    ps = pps.tile([p_out, gw], DT, tag="acc")
                    for (soff, sw) in ss:
                        a = goff + soff
                        for ki in range(kt):
                            nc.tensor.matmul(ps[:, soff:soff + sw],
                                             wtile(ki), src(ki)[:, a:a + sw],
                                             start=(ki == 0),
                                             stop=(ki == kt - 1))
                    ot = po.tile([p_out, gw], DT, tag="oS")
                    if mode == "eluclip":
                        elu_post(ps, gw, bias_col, ot[:, :], extra_min=10.0,
                                 part=p_out)
                    else:  # raw + bias
                        nc.scalar.activation(ot[:, :], ps[:, :gw], AF.Identity,
                                             bias=bias_col)
                    nc.gpsimd.dma_start(
                        out_dram[0:p_out, hoff + goff:hoff + goff + gw],
                        ot[:, :])

            hoff = 0
            for hf in [h for _ in range(passes) for h in (0, 1)]:
                bh = half_sizes[hf]
                groups = _chunk_groups(bh)
                ydh = pyd.tile([2, bh], R, tag="yd")
                nc.sync.dma_start(ydh[:, :], ydm[:, hoff:hoff + bh])
                wydt = pwyd.tile([2, 8, 128], R, tag="wyd")
                nc.sync.dma_start(wydt[:, :, :], zw0yd[:, :, :])
                zw0m0 = pw.tile([128, 8, 128], R, tag="W")
                nc.sync.dma_start(zw0m0[:, :, :], zw0[:, :, 0:128])
                xks = []
                for ki in range(8):
                    xk = px.tile([128, bh], R, tag=f"x{ki}")
                    nc.sync.dma_start(xk[:, :],
                                      xT[:, ki, hoff:hoff + bh])
                    xks.append(xk)

                xsrc = lambda ki: xks[ki][:, :]

                # ---- z branch ----
                hz1 = pact.tile([128, 8, bh], R, tag="hA")
                emit_layer(xsrc, 8, zw0, 8, zb0t,
                           lambda mi: hz1[:, mi, :], groups,
                           extra=lambda mi: (wydt[:, mi, :], ydh),
                           pre_w=zw0m0)
                hz2 = pact.tile([128, 8, bh], R, tag="hB")
                emit_layer(lambda ki: hz1[:, ki, :], 8, zw1, 8, zb1t,
                           lambda mi: hz2[:, mi, :], groups)

                # ---- z head (routed) ----
                zs = pzs.tile([128, 2, bh], DT, tag="zs")  # raw scale staging
                for (k, loff, n) in half_jobs[hf]:
                    for mj in range(4):
                        wt = pw.tile([128, 8, 128], R, tag="W")
                        nc.sync.dma_start(
                            wt[:, :, :],
                            zhW[k, :, :, 128 * mj:128 * (mj + 1)])
                        ps = pps.tile([128, n], DT, tag="acc")
                        for ki in range(8):
                            nc.tensor.matmul(ps[:, :],
                                             wt[:, ki, :],
                                             hz2[:, ki, loff:loff + n],
                                             start=(ki == 0), stop=(ki == 7))
                        bias = zhbt[:, 4 * k + mj:4 * k + mj + 1]
                        if mj < 2:
                            ot = po.tile([128, n], DT, tag="oS")
                            nc.vector.tensor_scalar(ot[:, :], ps[:, :],
                                                    bias, -100.0,
                                                    ALU.add, ALU.max)
                            nc.vector.tensor_scalar_min(ot[:, :], ot[:, :],
                                                        100.0)
                            nc.gpsimd.dma_start(
                                zloc[128 * mj:128 * (mj + 1),
                                     hoff + loff:hoff + loff + n], ot[:, :])
                        else:
                            nc.vector.tensor_scalar(
                                zs[:, mj - 2, loff:loff + n], ps[:, :],
                                bias, None, ALU.add)
                # ---- t branch ----
                h1 = pact.tile([128, 2, bh], R, tag="hA")
                emit_layer(xsrc, 8, tw0, 2, tb0t,
                           lambda mi: h1[:, mi, :], groups)
                h2 = pact.tile([128, 2, bh], R, tag="hB")
                emit_layer(lambda ki: h1[:, ki, :], 2, tw1, 2, tb1t,
                           lambda mi: h2[:, mi, :], groups)
                tw2t = pwyd.tile([128, 2, K], R, tag="tw2")
                nc.sync.dma_start(tw2t[:, :, :], tw2[:, :, :])
                emit_head(lambda ki: h2[:, ki, :], 2,
                          lambda ki: tw2t[:, ki, :], K, tb2t[:, 0:1],
                          tlog, hoff, groups, "eluclip")

                # ---- y branch ----
                hy1 = pact.tile([128, 8, bh], R, tag="hA")
                emit_layer(xsrc, 8, yw0, 8, yb0t,
                           lambda mi: hy1[:, mi, :], groups)
                hy2 = pact.tile([128, 8, bh], R, tag="hB")
                emit_layer(lambda ki: hy1[:, ki, :], 8, yw1, 8, yb1t,
                           lambda mi: hy2[:, mi, :], groups)
                emit_head(lambda ki: hy2[:, ki, :], 8,
                          lambda ki: yhwt[:, ki, :], 2 * K, yhbt[:, 0:1],
                          yall, hoff, groups, "raw")

                # batched softplus over the half's raw scale staging:
                # softplus(x) = relu(x) + ln(1 + exp(-|x|)).  Ops are phase-
                # ordered (all Relu/Abs/Exp, then all Ln) so the ACT table
                # switches exp<->ln at most once per half.
                all_units = [(mj, g) for mj in range(2) for g in groups]
                unit_chunks = [all_units[i:i + 2]
                               for i in range(0, len(all_units), 2)]
                for units in unit_chunks:
                  rts, e2s = [], []
                  for (mj, (goff, gw, _)) in units:
                    zsl = zs[:, mj, goff:goff + gw]
                    ot = po.tile([128, gw], DT, tag="oS")
                    nc.scalar.activation(ot[:, :], zsl, AF.Relu)
                    rts.append(ot)
                  for (mj, (goff, gw, _)) in units:
                    zsl = zs[:, mj, goff:goff + gw]
                    e1 = pe_.tile([128, gw], DT, tag="e")
                    nc.scalar.activation(e1[:, :], zsl, AF.Abs)
                    e2 = pe_.tile([128, gw], DT, tag="e")
                    nc.scalar.activation(e2[:, :], e1[:, :], AF.Exp,
                                         scale=-1.0)
                    e2s.append(e2)
                  for u, (mj, (goff, gw, _)) in enumerate(units):
                    zsl = zs[:, mj, goff:goff + gw]
                    nc.scalar.activation(zsl, e2s[u][:, :], AF.Ln, bias=1.0)
                    ot = rts[u]
                    nc.vector.tensor_tensor(ot[:, :], ot[:, :], zsl, ALU.add)
                    nc.vector.tensor_scalar(ot[:, :], ot[:, :], 0.001, 100.0,
                                            ALU.add, ALU.min)
                    nc.gpsimd.dma_start(
                        zscale[128 * mj:128 * (mj + 1),
                               hoff + goff:hoff + goff + gw], ot[:, :])
                # ---- d branch ----
                hd1 = pact.tile([128, 8, bh], R, tag="hA")
                emit_layer(xsrc, 8, dw0, 8, db0t,
                           lambda mi: hd1[:, mi, :], groups)
                hd2 = pact.tile([128, 8, bh], R, tag="hB")
                emit_layer(lambda ki: hd1[:, ki, :], 8, dw1, 8, db1t,
                           lambda mi: hd2[:, mi, :], groups)
                emit_head(lambda ki: hd2[:, ki, :], 8,
                          lambda ki: dhwt[:, ki, :], 2 * K, dhbt[:, 0:1],
                          dall, hoff, groups, "raw")

                hoff = (hoff + bh) % b_dev

    nc.compile()
    return nc


def _get_program(sig):
    if sig not in _PROG_CACHE:
        _PROG_CACHE[sig] = _build_program(sig)
    return _PROG_CACHE[sig]


# ----------------------------------------------------------------------------
# Host-side glue
# ----------------------------------------------------------------------------
def _softplus64(x):
    x = x.astype(np.float64)
    return np.log1p(np.exp(-np.abs(x))) + np.maximum(x, 0.0)


def _f32(a):
    return np.ascontiguousarray(np.asarray(a), dtype=np.float32)


def make_in_maps(inputs, plan):
    x = _f32(inputs["x"])
    y = _f32(inputs["y"])
    d = _f32(inputs["d"])
    perm = plan["perm"]

    def pack_bias(v, rows=128):
        v = _f32(v).reshape(-1)
        if len(v) % rows == 0 and len(v) >= rows:
            return np.ascontiguousarray(v.reshape(-1, rows).T)
        return v.reshape(-1, 1)

    # the device's hidden ELU outputs are shifted by +1 (fused op form);
    # compensate in every consumer's bias: b' = b - colsum(W)
    def csum(w):
        return np.asarray(w).astype(np.float64).sum(axis=0)

    def pack(w):
        """[kt*128, O] -> [128, kt, O] (partition-major weight layout)."""
        w = _f32(w)
        kt = w.shape[0] // 128
        return np.ascontiguousarray(
            w.reshape(kt, 128, w.shape[1]).transpose(1, 0, 2))

    yhw = _f32(np.transpose(np.asarray(inputs["yhW"]), (1, 0, 2))
               .reshape(H, 2 * K))
    dhw = _f32(np.transpose(np.asarray(inputs["dhW"]), (1, 0, 2))
               .reshape(H, 2 * K))
    zhW = _f32(inputs["zhW"])
    zhb_adj = (np.asarray(inputs["zhb"]).astype(np.float64)
               - zhW.astype(np.float64).sum(axis=1))  # [K, 512]
    zw0 = _f32(inputs["zw0"])
    shared = {
        "tw0": pack(inputs["tw0"]), "tw1": pack(inputs["tw1"]),
        "tw2": pack(inputs["tw2"]),
        "yw0": pack(inputs["yw0"]), "yw1": pack(inputs["yw1"]),
        "dw0": pack(inputs["dw0"]), "dw1": pack(inputs["dw1"]),
        "zw0": pack(zw0[2:]), "zw1": pack(inputs["zw1"]),
        "zw0yd": np.ascontiguousarray(zw0[:2].reshape(2, 8, 128)),
        "yhw": pack(yhw),
        "dhw": pack(dhw),
        "zhW": np.ascontiguousarray(
            zhW.reshape(K, 8, 128, 2 * L).transpose(0, 2, 1, 3)),
        "tb0": pack_bias(inputs["tb0"]),
        "tb1": pack_bias(np.asarray(inputs["tb1"]) - csum(inputs["tw1"])),
        "tb2": pack_bias(np.asarray(inputs["tb2"]) - csum(inputs["tw2"])),
        "yb0": pack_bias(inputs["yb0"]),
        "yb1": pack_bias(np.asarray(inputs["yb1"]) - csum(inputs["yw1"])),
        "db0": pack_bias(inputs["db0"]),
        "db1": pack_bias(np.asarray(inputs["db1"]) - csum(inputs["dw1"])),
        "zb0": pack_bias(inputs["zb0"]),
        "zb1": pack_bias(np.asarray(inputs["zb1"]) - csum(inputs["zw1"])),
        "yhb": _f32(np.asarray(inputs["yhb"]).reshape(2 * K)
                    - csum(yhw)).reshape(2 * K, 1),
        "dhb": _f32(np.asarray(inputs["dhb"]).reshape(2 * K)
                    - csum(dhw)).reshape(2 * K, 1),
        "zhb": np.ascontiguousarray(
            _f32(zhb_adj).reshape(K, 4, 128).transpose(2, 0, 1)
            .reshape(128, 4 * K)),
    }
    in_maps = []
    for c in range(NCORES):
        rows = perm[c]
        m = dict(shared)
        m["xT"] = np.ascontiguousarray(
            x[rows].T.reshape(8, 128, len(rows)).transpose(1, 0, 2))
        m["yd"] = np.ascontiguousarray(
            np.stack([y[rows], d[rows]], axis=0))
        in_maps.append(m)
    return in_maps


def assemble(results, plan, t):
    out = np.empty((B, K + 4 + 2 * L), dtype=np.float32)
    perm, valid = plan["perm"], plan["valid"]
    for c in range(NCORES):
        r = results[c]
        cols = np.nonzero(valid[c])[0]
        rows = perm[c][cols]
        out[rows, 0:K] = r["tlog"][:, cols].T
        tr = t[rows].astype(np.int64)
        ar = np.arange(len(cols))
        for name, o in (("yall", K), ("dall", K + 2)):
            a = r[name][:, cols]
            loc = np.clip(a[2 * tr, ar], -1e6, 1e6)
            scale = np.minimum(_softplus64(a[2 * tr + 1, ar]) + 1e-3, 1e6)
            out[rows, o] = loc
            out[rows, o + 1] = scale.astype(np.float32)
        out[rows, K + 4:K + 4 + L] = r["zloc"][:, cols].T
        out[rows, K + 4 + L:] = r["zscale"][:, cols].T
    return out


def kernel(**inputs) -> np.ndarray:
    t = np.asarray(inputs["t"]).astype(np.int32)
    plan = _plan_routing(t)
    sig = (plan["b_dev"], plan["half_sizes"], plan["half_jobs"])
    nc = _get_program(sig)
    in_maps = make_in_maps(inputs, plan)
    res = run_bass_kernel_spmd(nc, in_maps, list(range(NCORES)))
    return assemble(res.results, plan, t)
